# revision 14
# baseline (speedup 1.0000x reference)
"""GCNNet (3x GCNConv + mean/max pool + MLP head) on 8 Trainium2 NeuronCores.

Sharding: graph-parallel. 256 graphs -> 32 per core; each core owns its graphs'
nodes (dst side of every edge), padded to SLOT-aligned node slots so pooling
segments are fixed 128-column windows. Aggregation D^-1/2 (A+I') D^-1/2 @ h is
computed per 128-slot dst block as a chain of 128-edge scatter-matmuls:
  lhsT = h_pre rows gathered by indirect DMA from a replicated DRAM table,
  rhs  = per-chunk scatter matrix (dis[dst] at [edge, dstlocal]) -> PSUM.
Self-loop term dis^2*h is added at PSUM eviction. Feature-major transform
(W.T @ agg.T) + relu follows. Between convs the dis-scaled output is
PE-transposed to node-major and AllGathered (bf16) so every core can gather
any source row. Pooling = masked segmented reduce_sum/reduce_max; MLP head is
computed feature-major for the core's 32 graphs.

All data-dependent structure (chunk counts, scatter matrices, gather indices,
masks, 1/count) is computed on the host per call and baked into the build.
"""
import sys

sys.path.insert(0, "/opt/trn_rl_repo")

import numpy as np
import ml_dtypes

import concourse.bass as bass
import concourse.bacc as bacc
import concourse.mybir as mybir
import concourse.tile as tile
from concourse import bass_utils
from concourse.masks import make_identity

BF16 = ml_dtypes.bfloat16
P = 128
W = 8                      # cores
NG = 256                   # graphs
GPC = NG // W              # graphs per core
D = 128                    # node feature dim
SOLV = 512

RELU = mybir.ActivationFunctionType.Relu
COPY = mybir.ActivationFunctionType.Copy
F32 = mybir.dt.float32
BF = mybir.dt.bfloat16
I32 = mybir.dt.int32
I16 = mybir.dt.int16

WSHAPES = {"W1": (D, D), "W2": (D, 2 * D), "W3": (2 * D, 4 * D),
           "Wg1": (1024, 1024), "Wg2": (1024, 128), "Ws1": (512, 256),
           "Ws2": (256, 128), "Wf1": (256, 1024), "Wf2": (1024, 512),
           "Wo": (512, 128)}   # Wo zero-padded 1 -> 128 cols
BSHAPES = {"b1": D, "b2": 2 * D, "b3": 4 * D, "bg1": 1024, "bg2": 128,
           "bs1": 256, "bs2": 128, "bf1": 1024, "bf2": 512}


def _prep(x, edge_index, batch, solvent_fingerprint, weights):
    """Host-side structure computation. Returns (meta, per-core input maps)."""
    N = x.shape[0]
    src = np.asarray(edge_index[0], np.int64)
    dst = np.asarray(edge_index[1], np.int64)
    batch = np.asarray(batch, np.int64)

    deg = np.bincount(dst, minlength=N).astype(np.float64) + 1.0
    dis = (1.0 / np.sqrt(deg)).astype(np.float32)

    counts = np.bincount(batch, minlength=NG).astype(np.int64)
    maxn = max(int(counts.max()), 1)
    SLOT = max(((maxn + P - 1) // P) * P, P)
    NLOC = GPC * SLOT
    NBLK = NLOC // P
    NTAB = W * NLOC

    # node -> (core, local slot, global slot); rank = position within graph
    order = np.argsort(batch, kind="stable")
    rank = np.zeros(N, np.int64)
    start_of = np.zeros(NG + 1, np.int64)
    np.add.at(start_of[1:], batch, 1)
    start_of = np.cumsum(start_of)
    rank[order] = np.arange(N) - start_of[batch[order]]
    core_of = batch // GPC
    lslot = (batch - core_of * GPC) * SLOT + rank
    gslot = core_of * NLOC + lslot

    x_pre = np.asarray(x, np.float32) * dis[:, None]
    x_tab = np.zeros((NTAB, D), BF16)
    x_tab[gslot] = x_pre.astype(BF16)

    # edges grouped by (core, local dst block)
    e_core = core_of[dst]
    e_blk = lslot[dst] // P
    e_dstlocal = lslot[dst] % P
    e_key = e_core * NBLK + e_blk
    e_order = np.argsort(e_key, kind="stable")
    cb_counts = np.bincount(e_key, minlength=W * NBLK).reshape(W, NBLK)
    seg_start = np.concatenate([[0], np.cumsum(cb_counts.reshape(-1))])
    # dedup gathered src rows within each (core, block)
    uniq_all = {}
    u_counts = np.zeros((W, NBLK), np.int64)
    for c in range(W):
        for b in range(NBLK):
            k = c * NBLK + b
            ee = e_order[seg_start[k]:seg_start[k + 1]]
            if len(ee):
                uq, inv = np.unique(gslot[src[ee]], return_inverse=True)
                uniq_all[k] = (ee, uq, inv)
                u_counts[c, b] = len(uq)
    cnt = np.maximum((u_counts + P - 1) // P, 1).max(axis=0)   # chunks/block
    NCHUNK = int(cnt.sum())
    off = np.concatenate([[0], np.cumsum(cnt)]).astype(np.int64)

    src_idx = np.zeros((W, P, NCHUNK), np.int32)
    scatf = np.zeros((W, P, NCHUNK, P), np.float32)
    gx1 = np.zeros((W, P, NCHUNK, D), BF16)
    for c in range(W):
        for b in range(NBLK):
            k = c * NBLK + b
            if k not in uniq_all:
                continue
            ee, uq, inv = uniq_all[k]
            base = off[b]
            uch = np.arange(len(uq)) // P
            upp = np.arange(len(uq)) % P
            src_idx[c, upp, base + uch] = uq.astype(np.int32)
            gx1[c, upp, base + uch, :] = x_tab[uq]
            np.add.at(scatf[c], (inv % P, base + inv // P, e_dstlocal[ee]),
                      dis[dst[ee]])
    scat = scatf.astype(BF16)

    # dma_gather index layout: flat order is chunk-major (row i -> partition
    # i%128, chunk i//128), wrapped into 16 partitions (idx i at [i%16, i//16])
    # and replicated across the 8 Q7 cores' partition groups. int16.
    assert NTAB <= 32768, "dma_gather int16 indices"
    idx16 = np.zeros((W, P, NCHUNK * 8), np.int16)
    for c in range(W):
        flat = src_idx[c].T.reshape(-1)                  # [NCHUNK*128] chunk-major
        w16 = flat.reshape(-1, 16).T.astype(np.int16)    # [16, NCHUNK*8]
        idx16[c] = np.tile(w16, (8, 1))

    dis_loc = np.zeros((W, NLOC), np.float32)
    dis2_loc = np.zeros((W, NLOC), np.float32)
    mask_loc = np.zeros((W, NLOC), np.float32)
    xT_loc = np.zeros((W, D, NLOC), BF16)
    for c in range(W):
        m = core_of == c
        dis_loc[c, lslot[m]] = dis[m]
        dis2_loc[c, lslot[m]] = dis[m] ** 2
        mask_loc[c, lslot[m]] = 1.0
        xT_loc[c][:, lslot[m]] = x_pre[m].T.astype(BF16)

    inv_cnt = (1.0 / np.maximum(counts, 1)).astype(np.float32).reshape(W, GPC)
    sf = np.asarray(solvent_fingerprint, np.float32)
    sfT = sf.reshape(NG, SOLV).T.astype(BF16)

    meta = dict(SLOT=SLOT, NLOC=NLOC, NBLK=NBLK, NTAB=NTAB, NCHUNK=NCHUNK,
                cnt=[int(v) for v in cnt], off=[int(v) for v in off],
                scmax=int(cnt.max()))

    bb = np.broadcast_to
    in_maps = []
    for c in range(W):
        m = dict(
            gx1=np.ascontiguousarray(gx1[c]),
            idx16=np.ascontiguousarray(idx16[c]),
            scat=np.ascontiguousarray(scat[c]),
            xT_loc=np.ascontiguousarray(xT_loc[c]),
            disb=np.ascontiguousarray(bb(dis_loc[c][None, :], (P, NLOC)).astype(BF16)),
            dis2b=np.ascontiguousarray(bb(dis2_loc[c][None, :], (P, NLOC)).astype(BF16)),
            maskb=np.ascontiguousarray(bb(mask_loc[c][None, :], (P, NLOC)).astype(BF16)),
            invb=np.ascontiguousarray(bb(inv_cnt[c][None, :], (P, GPC)).astype(np.float32)),
            sfT=np.ascontiguousarray(sfT[:, c * GPC:(c + 1) * GPC]),
        )
        m.update(weights)
        in_maps.append(m)
    return meta, in_maps


def _build(meta, trace=False):
    import os
    PH = int(os.environ.get("KPH", "9"))
    SLOT, NLOC, NBLK, NTAB, NCHUNK = (meta["SLOT"], meta["NLOC"], meta["NBLK"],
                                      meta["NTAB"], meta["NCHUNK"])
    cnt, off, scmax = meta["cnt"], meta["off"], meta["scmax"]
    CPG = 4            # chunks per gather call (SWDGE ring: <=1024 descs/inst)
    NGRP = (NCHUNK + CPG - 1) // CPG

    nc = bacc.Bacc("TRN2", target_bir_lowering=False, debug=False, num_devices=W,
                   num_swdge_queues=4)

    gx1 = nc.dram_tensor("gx1", [P, NCHUNK, D], BF, kind="ExternalInput")
    idx16 = nc.dram_tensor("idx16", [P, NCHUNK * 8], I16, kind="ExternalInput")
    scat = nc.dram_tensor("scat", [P, NCHUNK, P], BF, kind="ExternalInput")
    xT_loc = nc.dram_tensor("xT_loc", [D, NLOC], BF, kind="ExternalInput")
    disb = nc.dram_tensor("disb", [P, NLOC], BF, kind="ExternalInput")
    dis2b = nc.dram_tensor("dis2b", [P, NLOC], BF, kind="ExternalInput")
    maskb = nc.dram_tensor("maskb", [P, NLOC], BF, kind="ExternalInput")
    invb = nc.dram_tensor("invb", [P, GPC], F32, kind="ExternalInput")
    sfT = nc.dram_tensor("sfT", [SOLV, GPC], BF, kind="ExternalInput")

    wdr = {k: nc.dram_tensor(k + "_bf", list(s), BF, kind="ExternalInput")
           for k, s in WSHAPES.items()}
    bdr = {k: nc.dram_tensor(k + "_f", [n, 1], F32, kind="ExternalInput")
           for k, n in BSHAPES.items()}
    out = nc.dram_tensor("out", [1, GPC], F32, kind="ExternalOutput")

    with tile.TileContext(nc) as tc:
        with tc.tile_pool(name="const", bufs=1) as cp, \
             tc.tile_pool(name="gath", bufs=2) as gp, \
             tc.tile_pool(name="scatp", bufs=2) as sp, \
             tc.tile_pool(name="selfp", bufs=2) as selfp, \
             tc.tile_pool(name="aggp", bufs=2) as aggp, \
             tc.tile_pool(name="hp", bufs=6) as hp, \
             tc.tile_pool(name="work", bufs=2) as wp, \
             tc.tile_pool(name="psA", bufs=2, space="PSUM") as psA, \
             tc.tile_pool(name="psB", bufs=2, space="PSUM") as psB, \
             tc.tile_pool(name="psT", bufs=2, space="PSUM") as psT, \
             tc.tile_pool(name="dram", bufs=1, space="DRAM") as dp:

            # ---- constants ----
            ident = cp.tile([P, P], BF)
            make_identity(nc, ident[:])
            w_sb = {}
            for name, (fi, fo) in WSHAPES.items():
                tiles = []
                for k in range(fi // P):
                    t = cp.tile([P, fo], BF, name=f"w_{name}_{k}")
                    nc.sync.dma_start(out=t[:], in_=wdr[name][k * P:(k + 1) * P, :])
                    tiles.append(t)
                w_sb[name] = tiles
            b_sb = {}
            for name, n in BSHAPES.items():
                tiles = []
                for m in range(n // P):
                    t = cp.tile([P, 1], F32, name=f"b_{name}_{m}")
                    nc.sync.dma_start(out=t[:], in_=bdr[name][m * P:(m + 1) * P, :])
                    tiles.append(t)
                b_sb[name] = tiles
            idx16_sb = cp.tile([P, NCHUNK * 8], I16)
            nc.sync.dma_start(out=idx16_sb[:], in_=idx16[:])
            disb_sb = cp.tile([P, NLOC], BF)
            nc.sync.dma_start(out=disb_sb[:], in_=disb[:])
            dis2b_sb = cp.tile([P, NLOC], BF)
            nc.sync.dma_start(out=dis2b_sb[:], in_=dis2b[:])
            maskb_sb = cp.tile([P, NLOC], BF)
            nc.sync.dma_start(out=maskb_sb[:], in_=maskb[:])
            invb_sb = cp.tile([P, GPC], F32)
            nc.sync.dma_start(out=invb_sb[:], in_=invb[:])
            sfT_sb = []
            for k in range(SOLV // P):
                t = cp.tile([P, GPC], BF, name=f"sfT_{k}")
                nc.sync.dma_start(out=t[:], in_=sfT[k * P:(k + 1) * P, :])
                sfT_sb.append(t)
            xT_sb = hp.tile([P, NLOC], BF, tag="hT", name="xT_sb")
            nc.sync.dma_start(out=xT_sb[:], in_=xT_loc[:])

            def conv(tag, table, f_in, f_out, w_name, b_name, hprevT,
                     pregathered=False, post_slice=None):
                """-> list of f_out//128 SBUF tiles [128, NLOC] bf16 (h.T)."""
                kt, mt = f_in // P, f_out // P
                selft = []
                for k in range(kt):
                    st = selfp.tile([P, NLOC], BF, tag="selft",
                                    name=f"self_{tag}_{k}")
                    nc.vector.tensor_tensor(out=st[:], in0=hprevT[k],
                                            in1=dis2b_sb[:],
                                            op=mybir.AluOpType.mult)
                    selft.append(st)
                aggT = [aggp.tile([P, NLOC], BF, tag="aggT",
                                  name=f"agg_{tag}_{k}") for k in range(kt)]
                with nc.named_scope(f"gath_{tag}"):
                    gtiles, stiles = {}, {}

                    def issue_group(g):
                        ch0, ch1 = g * CPG, min((g + 1) * CPG, NCHUNK)
                        nch = ch1 - ch0
                        gall = gp.tile([P, CPG, f_in], BF, tag="gall",
                                       name="gall", bufs=4)
                        if pregathered:
                            nc.sync.dma_start(out=gall[:, :nch, :],
                                              in_=table[:, ch0:ch1, :])
                        else:
                            nc.gpsimd.dma_gather(
                                gall[:, :nch, :], table[:],
                                idx16_sb[:, ch0 * 8:ch1 * 8],
                                nch * P, nch * P, f_in, queue_num=g % 4)
                        st = sp.tile([P, CPG * P], BF, tag="scat",
                                     name="scat_sb", bufs=4)
                        nc.sync.dma_start(out=st[:, :nch * P],
                                          in_=scat[:, ch0:ch1, :])
                        gtiles[g] = gall
                        stiles[g] = st

                    for b in range(NBLK):
                        cb = cnt[b]
                        for g in range(off[b] // CPG,
                                       (off[b] + cb - 1) // CPG + 1):
                            if g not in gtiles:
                                issue_group(g)
                        ps = [psA.tile([P, P], F32, space="PSUM",
                                       tag=f"ps{k}", name=f"ps{k}")
                              for k in range(kt)]
                        for c in range(cb):
                            ch = off[b] + c
                            g, col = ch // CPG, ch % CPG
                            for k in range(kt):
                                nc.tensor.matmul(
                                    out=ps[k][:],
                                    lhsT=gtiles[g][:, col, k * P:(k + 1) * P],
                                    rhs=stiles[g][:, col * P:(col + 1) * P],
                                    start=(c == 0), stop=(c == cb - 1))
                        for k in range(kt):
                            nc.vector.tensor_tensor(
                                out=aggT[k][:, b * P:(b + 1) * P],
                                in0=ps[k][:],
                                in1=selft[k][:, b * P:(b + 1) * P],
                                op=mybir.AluOpType.add)
                hT = [hp.tile([P, NLOC], BF, tag="hT", name=f"h_{tag}_{m}")
                      for m in range(mt)]
                with nc.named_scope(f"xf_{tag}"):
                    for m in range(mt):
                        for n0 in range(0, NLOC, 512):
                            ps2 = psB.tile([P, 512], F32, space="PSUM",
                                           tag="ps2", name="ps2")
                            for k in range(kt):
                                nc.tensor.matmul(
                                    out=ps2[:],
                                    lhsT=w_sb[w_name][k][:, m * P:(m + 1) * P],
                                    rhs=aggT[k][:, n0:n0 + 512],
                                    start=(k == 0), stop=(k == kt - 1))
                            nc.scalar.activation(
                                out=hT[m][:, n0:n0 + 512], in_=ps2[:], func=RELU,
                                bias=b_sb[b_name][m][:, 0:1])
                            if post_slice is not None:
                                post_slice(m, n0, hT[m])
                return hT

            def make_ag_sink(tag, f):
                ag_in = dp.tile([NLOC, f], BF, name=f"agin_{tag}")
                ag_out = dp.tile([NTAB, f], BF, addr_space="Shared",
                                 name=f"agout_{tag}")

                def post_slice(m, n0, hTm):
                    hpre = wp.tile([P, 512], BF, tag="hpre", name="hpre", bufs=3)
                    nc.vector.tensor_tensor(out=hpre[:], in0=hTm[:, n0:n0 + 512],
                                            in1=disb_sb[:, n0:n0 + 512],
                                            op=mybir.AluOpType.mult)
                    for j in range(4):
                        b = n0 // P + j
                        pst = psT.tile([P, P], BF, space="PSUM", tag="pst",
                                       name="pst")
                        nc.tensor.transpose(out=pst[:],
                                            in_=hpre[:, j * P:(j + 1) * P],
                                            identity=ident[:])
                        ev = wp.tile([P, P], BF, tag="ev", name="ev", bufs=3)
                        nc.scalar.activation(out=ev[:], in_=pst[:], func=COPY)
                        nc.sync.dma_start(
                            out=ag_in[b * P:(b + 1) * P, m * P:(m + 1) * P],
                            in_=ev[:])

                def finish():
                    with nc.named_scope(f"ag_{tag}"):
                        nc.gpsimd.collective_compute(
                            "AllGather", mybir.AluOpType.bypass,
                            replica_groups=[list(range(W))],
                            ins=[ag_in[:]], outs=[ag_out[:]])
                    return ag_out
                return post_slice, finish

            # ---- conv stack ----
            sink1, fin1 = make_ag_sink("t1", D)
            h1T = conv("c1", gx1, D, D, "W1", "b1", [xT_sb[:]],
                       pregathered=True, post_slice=sink1)
            tab1 = fin1()
            sink2, fin2 = make_ag_sink("t2", 2 * D)
            h2T = conv("c2", tab1, D, 2 * D, "W2", "b2",
                       [t[:] for t in h1T], post_slice=sink2)
            tab2 = fin2()

            def mask_slice(m, n0, hTm):
                nc.vector.tensor_tensor(out=hTm[:, n0:n0 + 512],
                                        in0=hTm[:, n0:n0 + 512],
                                        in1=maskb_sb[:, n0:n0 + 512],
                                        op=mybir.AluOpType.mult)
            h3T = conv("c3", tab2, 2 * D, 4 * D, "W3", "b3",
                       [t[:] for t in h2T], post_slice=mask_slice)

            # ---- pooling ----
            gapT = wp.tile([P, 4, GPC], F32, name="gapT")
            gmpT = wp.tile([P, 4, GPC], F32, name="gmpT")
            with nc.named_scope("pool"):
                for m in range(4):
                    seg3 = h3T[m][:].rearrange("p (g s) -> p g s", s=SLOT)
                    nc.vector.reduce_sum(out=gapT[:, m, :], in_=seg3,
                                         axis=mybir.AxisListType.X)
                    nc.vector.reduce_max(out=gmpT[:, m, :], in_=seg3,
                                         axis=mybir.AxisListType.X)

            # ---- head ----
            def dense(tag, rhs_aps, w_name, b_name, f_in, f_out):
                kt, mt = f_in // P, f_out // P
                outs = []
                for m in range(mt):
                    ps2 = psB.tile([P, GPC], F32, space="PSUM", tag="ps2",
                                   name="ps2")
                    for k in range(kt):
                        nc.tensor.matmul(
                            out=ps2[:],
                            lhsT=w_sb[w_name][k][:, m * P:(m + 1) * P],
                            rhs=rhs_aps[k], start=(k == 0), stop=(k == kt - 1))
                    o = wp.tile([P, GPC], BF, tag=f"hd_{tag}_{m}", name=f"hd_{tag}_{m}")
                    nc.scalar.activation(out=o[:], in_=ps2[:], func=RELU,
                                         bias=b_sb[b_name][m][:, 0:1])
                    outs.append(o[:])
                return outs

            with nc.named_scope("head"):
                gcat = []
                for m in range(4):
                    t = wp.tile([P, GPC], BF, tag=f"gap_{m}", name=f"gap_{m}")
                    nc.vector.tensor_tensor(out=t[:], in0=gapT[:, m, :],
                                            in1=invb_sb[:],
                                            op=mybir.AluOpType.mult)
                    gcat.append(t[:])
                for m in range(4):
                    t = wp.tile([P, GPC], BF, tag=f"gmp_{m}", name=f"gmp_{m}")
                    nc.vector.tensor_copy(out=t[:], in_=gmpT[:, m, :])
                    gcat.append(t[:])
                g1 = dense("g1", gcat, "Wg1", "bg1", 1024, 1024)
                g2 = dense("g2", g1, "Wg2", "bg2", 1024, 128)
                s1 = dense("s1", [t[:] for t in sfT_sb], "Ws1", "bs1", 512, 256)
                s2 = dense("s2", s1, "Ws2", "bs2", 256, 128)
                f1 = dense("f1", g2 + s2, "Wf1", "bf1", 256, 1024)
                f2 = dense("f2", f1, "Wf2", "bf2", 1024, 512)
                pso = psB.tile([P, GPC], F32, space="PSUM", tag="ps2", name="pso")
                for k in range(4):
                    nc.tensor.matmul(out=pso[:], lhsT=w_sb["Wo"][k][:],
                                     rhs=f2[k], start=(k == 0), stop=(k == 3))
                oo = wp.tile([1, GPC], F32, name="oo")
                nc.scalar.activation(out=oo[:], in_=pso[0:1, :], func=COPY,
                                     bias=float(meta["bo"]))
                nc.sync.dma_start(out=out[:], in_=oo[:])

    nc.compile()
    return nc


def kernel(**inputs):
    x = np.asarray(inputs["x"], np.float32)
    edge_index = np.asarray(inputs["edge_index"])
    batch = np.asarray(inputs["batch"])
    sf = np.asarray(inputs["solvent_fingerprint"], np.float32)

    weights = {}
    for k in WSHAPES:
        wv = np.asarray(inputs[k], np.float32)
        if k == "Wo":                       # pad [512,1] -> [512,128]
            wv = np.concatenate([wv, np.zeros((512, 127), np.float32)], axis=1)
        weights[k + "_bf"] = np.ascontiguousarray(wv.astype(BF16))
    for k in BSHAPES:
        weights[k + "_f"] = np.ascontiguousarray(
            np.asarray(inputs[k], np.float32).reshape(-1, 1))

    meta, in_maps = _prep(x, edge_index, batch, sf, weights)
    meta["bo"] = float(np.asarray(inputs["bo"]).reshape(-1)[0])
    nc = _build(meta)
    res = bass_utils.run_bass_kernel_spmd(nc, in_maps, core_ids=list(range(W)))
    out = np.zeros((NG, 1), np.float32)
    for c in range(W):
        out[c * GPC:(c + 1) * GPC, 0] = res.results[c]["out"][0]
    return out


# exposed for test.py: run with tracing and return (out, results)
def kernel_traced(**inputs):
    x = np.asarray(inputs["x"], np.float32)
    edge_index = np.asarray(inputs["edge_index"])
    batch = np.asarray(inputs["batch"])
    sf = np.asarray(inputs["solvent_fingerprint"], np.float32)
    weights = {}
    for k in WSHAPES:
        wv = np.asarray(inputs[k], np.float32)
        if k == "Wo":
            wv = np.concatenate([wv, np.zeros((512, 127), np.float32)], axis=1)
        weights[k + "_bf"] = np.ascontiguousarray(wv.astype(BF16))
    for k in BSHAPES:
        weights[k + "_f"] = np.ascontiguousarray(
            np.asarray(inputs[k], np.float32).reshape(-1, 1))
    meta, in_maps = _prep(x, edge_index, batch, sf, weights)
    meta["bo"] = float(np.asarray(inputs["bo"]).reshape(-1)[0])
    nc = _build(meta)
    res = bass_utils.run_bass_kernel_spmd(nc, in_maps, core_ids=list(range(W)),
                                          trace=True)
    out = np.zeros((NG, 1), np.float32)
    for c in range(W):
        out[c * GPC:(c + 1) * GPC, 0] = res.results[c]["out"][0]
    return out, res



# revision 15
# speedup vs baseline: 1.0708x; 1.0708x over previous
"""GCNNet (3x GCNConv + mean/max pool + MLP head) on 8 Trainium2 NeuronCores.

Sharding: graph-parallel. 256 graphs -> 32 per core; each core owns its graphs'
nodes (dst side of every edge), padded to SLOT-aligned node slots so pooling
segments are fixed 128-column windows. Aggregation D^-1/2 (A+I') D^-1/2 @ h is
computed per 128-slot dst block as a chain of 128-edge scatter-matmuls:
  lhsT = h_pre rows gathered by indirect DMA from a replicated DRAM table,
  rhs  = per-chunk scatter matrix (dis[dst] at [edge, dstlocal]) -> PSUM.
Self-loop term dis^2*h is added at PSUM eviction. Feature-major transform
(W.T @ agg.T) + relu follows. Between convs the dis-scaled output is
PE-transposed to node-major and AllGathered (bf16) so every core can gather
any source row. Pooling = masked segmented reduce_sum/reduce_max; MLP head is
computed feature-major for the core's 32 graphs.

All data-dependent structure (chunk counts, scatter matrices, gather indices,
masks, 1/count) is computed on the host per call and baked into the build.
"""
import sys

sys.path.insert(0, "/opt/trn_rl_repo")

import numpy as np
import ml_dtypes

import concourse.bass as bass
import concourse.bacc as bacc
import concourse.mybir as mybir
import concourse.tile as tile
from concourse import bass_utils
from concourse.masks import make_identity

BF16 = ml_dtypes.bfloat16
P = 128
W = 8                      # cores
NG = 256                   # graphs
GPC = NG // W              # graphs per core
D = 128                    # node feature dim
SOLV = 512

RELU = mybir.ActivationFunctionType.Relu
COPY = mybir.ActivationFunctionType.Copy
F32 = mybir.dt.float32
BF = mybir.dt.bfloat16
I32 = mybir.dt.int32
I16 = mybir.dt.int16

WSHAPES = {"W1": (D, D), "W2": (D, 2 * D), "W3": (2 * D, 4 * D),
           "Wg1": (1024, 1024), "Wg2": (1024, 128), "Ws1": (512, 256),
           "Ws2": (256, 128), "Wf1": (256, 1024), "Wf2": (1024, 512),
           "Wo": (512, 128)}   # Wo zero-padded 1 -> 128 cols
BSHAPES = {"b1": D, "b2": 2 * D, "b3": 4 * D, "bg1": 1024, "bg2": 128,
           "bs1": 256, "bs2": 128, "bf1": 1024, "bf2": 512}


def _prep(x, edge_index, batch, solvent_fingerprint, weights):
    """Host-side structure computation. Returns (meta, per-core input maps)."""
    N = x.shape[0]
    src = np.asarray(edge_index[0], np.int64)
    dst = np.asarray(edge_index[1], np.int64)
    batch = np.asarray(batch, np.int64)

    deg = np.bincount(dst, minlength=N).astype(np.float64) + 1.0
    dis = (1.0 / np.sqrt(deg)).astype(np.float32)

    counts = np.bincount(batch, minlength=NG).astype(np.int64)
    maxn = max(int(counts.max()), 1)
    SLOT = max(((maxn + P - 1) // P) * P, P)
    NLOC = GPC * SLOT
    NBLK = NLOC // P
    NTAB = W * NLOC

    # node -> (core, local slot, global slot); rank = position within graph
    order = np.argsort(batch, kind="stable")
    rank = np.zeros(N, np.int64)
    start_of = np.zeros(NG + 1, np.int64)
    np.add.at(start_of[1:], batch, 1)
    start_of = np.cumsum(start_of)
    rank[order] = np.arange(N) - start_of[batch[order]]
    core_of = batch // GPC
    lslot = (batch - core_of * GPC) * SLOT + rank
    gslot = core_of * NLOC + lslot

    x_pre = np.asarray(x, np.float32) * dis[:, None]
    x_tab = np.zeros((NTAB, D), BF16)
    x_tab[gslot] = x_pre.astype(BF16)

    # edges grouped by (core, local dst block)
    e_core = core_of[dst]
    e_blk = lslot[dst] // P
    e_dstlocal = lslot[dst] % P
    e_key = e_core * NBLK + e_blk
    e_order = np.argsort(e_key, kind="stable")
    cb_counts = np.bincount(e_key, minlength=W * NBLK).reshape(W, NBLK)
    seg_start = np.concatenate([[0], np.cumsum(cb_counts.reshape(-1))])
    # dedup gathered src rows within each (core, block)
    uniq_all = {}
    u_counts = np.zeros((W, NBLK), np.int64)
    for c in range(W):
        for b in range(NBLK):
            k = c * NBLK + b
            ee = e_order[seg_start[k]:seg_start[k + 1]]
            if len(ee):
                uq, inv = np.unique(gslot[src[ee]], return_inverse=True)
                uniq_all[k] = (ee, uq, inv)
                u_counts[c, b] = len(uq)
    cnt = np.maximum((u_counts + P - 1) // P, 1).max(axis=0)   # chunks/block
    NCHUNK = int(cnt.sum())
    off = np.concatenate([[0], np.cumsum(cnt)]).astype(np.int64)

    src_idx = np.zeros((W, P, NCHUNK), np.int32)
    scatf = np.zeros((W, P, NCHUNK, P), np.float32)
    gx1 = np.zeros((W, P, NCHUNK, D), BF16)
    for c in range(W):
        for b in range(NBLK):
            k = c * NBLK + b
            if k not in uniq_all:
                continue
            ee, uq, inv = uniq_all[k]
            base = off[b]
            uch = np.arange(len(uq)) // P
            upp = np.arange(len(uq)) % P
            src_idx[c, upp, base + uch] = uq.astype(np.int32)
            gx1[c, upp, base + uch, :] = x_tab[uq]
            np.add.at(scatf[c], (inv % P, base + inv // P, e_dstlocal[ee]),
                      dis[dst[ee]])
    scat = scatf.astype(BF16)

    # dma_gather index layout: flat order is chunk-major (row i -> partition
    # i%128, chunk i//128), wrapped into 16 partitions (idx i at [i%16, i//16])
    # and replicated across the 8 Q7 cores' partition groups. int16.
    assert NTAB <= 32768, "dma_gather int16 indices"
    idx16 = np.zeros((W, P, NCHUNK * 8), np.int16)
    for c in range(W):
        flat = src_idx[c].T.reshape(-1)                  # [NCHUNK*128] chunk-major
        w16 = flat.reshape(-1, 16).T.astype(np.int16)    # [16, NCHUNK*8]
        idx16[c] = np.tile(w16, (8, 1))

    dis_loc = np.zeros((W, NLOC), np.float32)
    dis2_loc = np.zeros((W, NLOC), np.float32)
    mask_loc = np.zeros((W, NLOC), np.float32)
    xT_loc = np.zeros((W, D, NLOC), BF16)
    for c in range(W):
        m = core_of == c
        dis_loc[c, lslot[m]] = dis[m]
        dis2_loc[c, lslot[m]] = dis[m] ** 2
        mask_loc[c, lslot[m]] = 1.0
        xT_loc[c][:, lslot[m]] = x_pre[m].T.astype(BF16)

    inv_cnt = (1.0 / np.maximum(counts, 1)).astype(np.float32).reshape(W, GPC)
    sf = np.asarray(solvent_fingerprint, np.float32)
    sfT = sf.reshape(NG, SOLV).T.astype(BF16)

    meta = dict(SLOT=SLOT, NLOC=NLOC, NBLK=NBLK, NTAB=NTAB, NCHUNK=NCHUNK,
                cnt=[int(v) for v in cnt], off=[int(v) for v in off],
                scmax=int(cnt.max()))

    bb = np.broadcast_to
    in_maps = []
    for c in range(W):
        m = dict(
            gx1=np.ascontiguousarray(gx1[c]),
            idx16=np.ascontiguousarray(idx16[c]),
            scat=np.ascontiguousarray(scat[c]),
            xT_loc=np.ascontiguousarray(xT_loc[c]),
            disb=np.ascontiguousarray(bb(dis_loc[c][None, :], (P, NLOC)).astype(BF16)),
            dis2b=np.ascontiguousarray(bb(dis2_loc[c][None, :], (P, NLOC)).astype(BF16)),
            maskb=np.ascontiguousarray(bb(mask_loc[c][None, :], (P, NLOC)).astype(BF16)),
            invb=np.ascontiguousarray(bb(inv_cnt[c][None, :], (P, GPC)).astype(np.float32)),
            sfT=np.ascontiguousarray(sfT[:, c * GPC:(c + 1) * GPC]),
        )
        m.update(weights)
        in_maps.append(m)
    return meta, in_maps


def _build(meta, trace=False):
    import os
    PH = int(os.environ.get("KPH", "9"))
    SLOT, NLOC, NBLK, NTAB, NCHUNK = (meta["SLOT"], meta["NLOC"], meta["NBLK"],
                                      meta["NTAB"], meta["NCHUNK"])
    cnt, off, scmax = meta["cnt"], meta["off"], meta["scmax"]
    CPG = 8            # chunks per gather call (SWDGE ring: <=1024 descs/inst)
    NGRP = (NCHUNK + CPG - 1) // CPG

    nc = bacc.Bacc("TRN2", target_bir_lowering=False, debug=False, num_devices=W,
                   num_swdge_queues=4, dynamic_dma_scratch_size=32768)

    gx1 = nc.dram_tensor("gx1", [P, NCHUNK, D], BF, kind="ExternalInput")
    idx16 = nc.dram_tensor("idx16", [P, NCHUNK * 8], I16, kind="ExternalInput")
    scat = nc.dram_tensor("scat", [P, NCHUNK, P], BF, kind="ExternalInput")
    xT_loc = nc.dram_tensor("xT_loc", [D, NLOC], BF, kind="ExternalInput")
    disb = nc.dram_tensor("disb", [P, NLOC], BF, kind="ExternalInput")
    dis2b = nc.dram_tensor("dis2b", [P, NLOC], BF, kind="ExternalInput")
    maskb = nc.dram_tensor("maskb", [P, NLOC], BF, kind="ExternalInput")
    invb = nc.dram_tensor("invb", [P, GPC], F32, kind="ExternalInput")
    sfT = nc.dram_tensor("sfT", [SOLV, GPC], BF, kind="ExternalInput")

    wdr = {k: nc.dram_tensor(k + "_bf", list(s), BF, kind="ExternalInput")
           for k, s in WSHAPES.items()}
    bdr = {k: nc.dram_tensor(k + "_f", [n, 1], F32, kind="ExternalInput")
           for k, n in BSHAPES.items()}
    out = nc.dram_tensor("out", [1, GPC], F32, kind="ExternalOutput")

    with tile.TileContext(nc) as tc:
        with tc.tile_pool(name="const", bufs=1) as cp, \
             tc.tile_pool(name="gath", bufs=2) as gp, \
             tc.tile_pool(name="scatp", bufs=2) as sp, \
             tc.tile_pool(name="selfp", bufs=2) as selfp, \
             tc.tile_pool(name="aggp", bufs=2) as aggp, \
             tc.tile_pool(name="hp", bufs=6) as hp, \
             tc.tile_pool(name="work", bufs=2) as wp, \
             tc.tile_pool(name="psA", bufs=2, space="PSUM") as psA, \
             tc.tile_pool(name="psB", bufs=2, space="PSUM") as psB, \
             tc.tile_pool(name="psT", bufs=2, space="PSUM") as psT, \
             tc.tile_pool(name="dram", bufs=1, space="DRAM") as dp:

            # ---- constants ----
            ident = cp.tile([P, P], BF)
            make_identity(nc, ident[:])
            w_sb = {}
            for name, (fi, fo) in WSHAPES.items():
                tiles = []
                for k in range(fi // P):
                    t = cp.tile([P, fo], BF, name=f"w_{name}_{k}")
                    nc.sync.dma_start(out=t[:], in_=wdr[name][k * P:(k + 1) * P, :])
                    tiles.append(t)
                w_sb[name] = tiles
            b_sb = {}
            for name, n in BSHAPES.items():
                tiles = []
                for m in range(n // P):
                    t = cp.tile([P, 1], F32, name=f"b_{name}_{m}")
                    nc.sync.dma_start(out=t[:], in_=bdr[name][m * P:(m + 1) * P, :])
                    tiles.append(t)
                b_sb[name] = tiles
            idx16_sb = cp.tile([P, NCHUNK * 8], I16)
            nc.sync.dma_start(out=idx16_sb[:], in_=idx16[:])
            disb_sb = cp.tile([P, NLOC], BF)
            nc.sync.dma_start(out=disb_sb[:], in_=disb[:])
            dis2b_sb = cp.tile([P, NLOC], BF)
            nc.sync.dma_start(out=dis2b_sb[:], in_=dis2b[:])
            maskb_sb = cp.tile([P, NLOC], BF)
            nc.sync.dma_start(out=maskb_sb[:], in_=maskb[:])
            invb_sb = cp.tile([P, GPC], F32)
            nc.sync.dma_start(out=invb_sb[:], in_=invb[:])
            sfT_sb = []
            for k in range(SOLV // P):
                t = cp.tile([P, GPC], BF, name=f"sfT_{k}")
                nc.sync.dma_start(out=t[:], in_=sfT[k * P:(k + 1) * P, :])
                sfT_sb.append(t)
            xT_sb = hp.tile([P, NLOC], BF, tag="hT", name="xT_sb")
            nc.sync.dma_start(out=xT_sb[:], in_=xT_loc[:])

            def conv(tag, table, f_in, f_out, w_name, b_name, hprevT,
                     pregathered=False, post_slice=None):
                """-> list of f_out//128 SBUF tiles [128, NLOC] bf16 (h.T)."""
                kt, mt = f_in // P, f_out // P
                selft = []
                for k in range(kt):
                    st = selfp.tile([P, NLOC], BF, tag="selft",
                                    name=f"self_{tag}_{k}")
                    nc.vector.tensor_tensor(out=st[:], in0=hprevT[k],
                                            in1=dis2b_sb[:],
                                            op=mybir.AluOpType.mult)
                    selft.append(st)
                aggT = [aggp.tile([P, NLOC], BF, tag="aggT",
                                  name=f"agg_{tag}_{k}") for k in range(kt)]
                with nc.named_scope(f"gath_{tag}"):
                    gtiles, stiles = {}, {}

                    def issue_group(g):
                        ch0, ch1 = g * CPG, min((g + 1) * CPG, NCHUNK)
                        nch = ch1 - ch0
                        gall = gp.tile([P, CPG, f_in], BF, tag="gall",
                                       name="gall", bufs=4)
                        if pregathered:
                            nc.sync.dma_start(out=gall[:, :nch, :],
                                              in_=table[:, ch0:ch1, :])
                        else:
                            nc.gpsimd.dma_gather(
                                gall[:, :nch, :], table[:],
                                idx16_sb[:, ch0 * 8:ch1 * 8],
                                nch * P, nch * P, f_in, queue_num=g % 4)
                        st = sp.tile([P, CPG * P], BF, tag="scat",
                                     name="scat_sb", bufs=4)
                        nc.sync.dma_start(out=st[:, :nch * P],
                                          in_=scat[:, ch0:ch1, :])
                        gtiles[g] = gall
                        stiles[g] = st

                    for b in range(NBLK):
                        cb = cnt[b]
                        for g in range(off[b] // CPG,
                                       (off[b] + cb - 1) // CPG + 1):
                            if g not in gtiles:
                                issue_group(g)
                        ps = [psA.tile([P, P], F32, space="PSUM",
                                       tag=f"ps{k}", name=f"ps{k}")
                              for k in range(kt)]
                        for c in range(cb):
                            ch = off[b] + c
                            g, col = ch // CPG, ch % CPG
                            for k in range(kt):
                                nc.tensor.matmul(
                                    out=ps[k][:],
                                    lhsT=gtiles[g][:, col, k * P:(k + 1) * P],
                                    rhs=stiles[g][:, col * P:(col + 1) * P],
                                    start=(c == 0), stop=(c == cb - 1))
                        for k in range(kt):
                            nc.vector.tensor_tensor(
                                out=aggT[k][:, b * P:(b + 1) * P],
                                in0=ps[k][:],
                                in1=selft[k][:, b * P:(b + 1) * P],
                                op=mybir.AluOpType.add)
                hT = [hp.tile([P, NLOC], BF, tag="hT", name=f"h_{tag}_{m}")
                      for m in range(mt)]
                with nc.named_scope(f"xf_{tag}"):
                    for m in range(mt):
                        for n0 in range(0, NLOC, 512):
                            ps2 = psB.tile([P, 512], F32, space="PSUM",
                                           tag="ps2", name="ps2")
                            for k in range(kt):
                                nc.tensor.matmul(
                                    out=ps2[:],
                                    lhsT=w_sb[w_name][k][:, m * P:(m + 1) * P],
                                    rhs=aggT[k][:, n0:n0 + 512],
                                    start=(k == 0), stop=(k == kt - 1))
                            nc.scalar.activation(
                                out=hT[m][:, n0:n0 + 512], in_=ps2[:], func=RELU,
                                bias=b_sb[b_name][m][:, 0:1])
                            if post_slice is not None:
                                post_slice(m, n0, hT[m])
                return hT

            def make_ag_sink(tag, f):
                ag_in = dp.tile([NLOC, f], BF, name=f"agin_{tag}")
                ag_out = dp.tile([NTAB, f], BF, addr_space="Shared",
                                 name=f"agout_{tag}")

                def post_slice(m, n0, hTm):
                    hpre = wp.tile([P, 512], BF, tag="hpre", name="hpre", bufs=3)
                    nc.vector.tensor_tensor(out=hpre[:], in0=hTm[:, n0:n0 + 512],
                                            in1=disb_sb[:, n0:n0 + 512],
                                            op=mybir.AluOpType.mult)
                    for j in range(4):
                        b = n0 // P + j
                        pst = psT.tile([P, P], BF, space="PSUM", tag="pst",
                                       name="pst")
                        nc.tensor.transpose(out=pst[:],
                                            in_=hpre[:, j * P:(j + 1) * P],
                                            identity=ident[:])
                        ev = wp.tile([P, P], BF, tag="ev", name="ev", bufs=3)
                        nc.scalar.activation(out=ev[:], in_=pst[:], func=COPY)
                        nc.sync.dma_start(
                            out=ag_in[b * P:(b + 1) * P, m * P:(m + 1) * P],
                            in_=ev[:])

                def finish():
                    with nc.named_scope(f"ag_{tag}"):
                        nc.gpsimd.collective_compute(
                            "AllGather", mybir.AluOpType.bypass,
                            replica_groups=[list(range(W))],
                            ins=[ag_in[:]], outs=[ag_out[:]])
                    return ag_out
                return post_slice, finish

            # ---- conv stack ----
            sink1, fin1 = make_ag_sink("t1", D)
            h1T = conv("c1", gx1, D, D, "W1", "b1", [xT_sb[:]],
                       pregathered=True, post_slice=sink1)
            tab1 = fin1()
            sink2, fin2 = make_ag_sink("t2", 2 * D)
            h2T = conv("c2", tab1, D, 2 * D, "W2", "b2",
                       [t[:] for t in h1T], post_slice=sink2)
            tab2 = fin2()

            def mask_slice(m, n0, hTm):
                nc.vector.tensor_tensor(out=hTm[:, n0:n0 + 512],
                                        in0=hTm[:, n0:n0 + 512],
                                        in1=maskb_sb[:, n0:n0 + 512],
                                        op=mybir.AluOpType.mult)
            h3T = conv("c3", tab2, 2 * D, 4 * D, "W3", "b3",
                       [t[:] for t in h2T], post_slice=mask_slice)

            # ---- pooling ----
            gapT = wp.tile([P, 4, GPC], F32, name="gapT")
            gmpT = wp.tile([P, 4, GPC], F32, name="gmpT")
            with nc.named_scope("pool"):
                for m in range(4):
                    seg3 = h3T[m][:].rearrange("p (g s) -> p g s", s=SLOT)
                    nc.vector.reduce_sum(out=gapT[:, m, :], in_=seg3,
                                         axis=mybir.AxisListType.X)
                    nc.vector.reduce_max(out=gmpT[:, m, :], in_=seg3,
                                         axis=mybir.AxisListType.X)

            # ---- head ----
            def dense(tag, rhs_aps, w_name, b_name, f_in, f_out):
                kt, mt = f_in // P, f_out // P
                outs = []
                for m in range(mt):
                    ps2 = psB.tile([P, GPC], F32, space="PSUM", tag="ps2",
                                   name="ps2")
                    for k in range(kt):
                        nc.tensor.matmul(
                            out=ps2[:],
                            lhsT=w_sb[w_name][k][:, m * P:(m + 1) * P],
                            rhs=rhs_aps[k], start=(k == 0), stop=(k == kt - 1))
                    o = wp.tile([P, GPC], BF, tag=f"hd_{tag}_{m}", name=f"hd_{tag}_{m}")
                    nc.scalar.activation(out=o[:], in_=ps2[:], func=RELU,
                                         bias=b_sb[b_name][m][:, 0:1])
                    outs.append(o[:])
                return outs

            with nc.named_scope("head"):
                gcat = []
                for m in range(4):
                    t = wp.tile([P, GPC], BF, tag=f"gap_{m}", name=f"gap_{m}")
                    nc.vector.tensor_tensor(out=t[:], in0=gapT[:, m, :],
                                            in1=invb_sb[:],
                                            op=mybir.AluOpType.mult)
                    gcat.append(t[:])
                for m in range(4):
                    t = wp.tile([P, GPC], BF, tag=f"gmp_{m}", name=f"gmp_{m}")
                    nc.vector.tensor_copy(out=t[:], in_=gmpT[:, m, :])
                    gcat.append(t[:])
                g1 = dense("g1", gcat, "Wg1", "bg1", 1024, 1024)
                g2 = dense("g2", g1, "Wg2", "bg2", 1024, 128)
                s1 = dense("s1", [t[:] for t in sfT_sb], "Ws1", "bs1", 512, 256)
                s2 = dense("s2", s1, "Ws2", "bs2", 256, 128)
                f1 = dense("f1", g2 + s2, "Wf1", "bf1", 256, 1024)
                f2 = dense("f2", f1, "Wf2", "bf2", 1024, 512)
                pso = psB.tile([P, GPC], F32, space="PSUM", tag="ps2", name="pso")
                for k in range(4):
                    nc.tensor.matmul(out=pso[:], lhsT=w_sb["Wo"][k][:],
                                     rhs=f2[k], start=(k == 0), stop=(k == 3))
                oo = wp.tile([1, GPC], F32, name="oo")
                nc.scalar.activation(out=oo[:], in_=pso[0:1, :], func=COPY,
                                     bias=float(meta["bo"]))
                nc.sync.dma_start(out=out[:], in_=oo[:])

    nc.compile()
    return nc


def kernel(**inputs):
    x = np.asarray(inputs["x"], np.float32)
    edge_index = np.asarray(inputs["edge_index"])
    batch = np.asarray(inputs["batch"])
    sf = np.asarray(inputs["solvent_fingerprint"], np.float32)

    weights = {}
    for k in WSHAPES:
        wv = np.asarray(inputs[k], np.float32)
        if k == "Wo":                       # pad [512,1] -> [512,128]
            wv = np.concatenate([wv, np.zeros((512, 127), np.float32)], axis=1)
        weights[k + "_bf"] = np.ascontiguousarray(wv.astype(BF16))
    for k in BSHAPES:
        weights[k + "_f"] = np.ascontiguousarray(
            np.asarray(inputs[k], np.float32).reshape(-1, 1))

    meta, in_maps = _prep(x, edge_index, batch, sf, weights)
    meta["bo"] = float(np.asarray(inputs["bo"]).reshape(-1)[0])
    nc = _build(meta)
    res = bass_utils.run_bass_kernel_spmd(nc, in_maps, core_ids=list(range(W)))
    out = np.zeros((NG, 1), np.float32)
    for c in range(W):
        out[c * GPC:(c + 1) * GPC, 0] = res.results[c]["out"][0]
    return out


# exposed for test.py: run with tracing and return (out, results)
def kernel_traced(**inputs):
    x = np.asarray(inputs["x"], np.float32)
    edge_index = np.asarray(inputs["edge_index"])
    batch = np.asarray(inputs["batch"])
    sf = np.asarray(inputs["solvent_fingerprint"], np.float32)
    weights = {}
    for k in WSHAPES:
        wv = np.asarray(inputs[k], np.float32)
        if k == "Wo":
            wv = np.concatenate([wv, np.zeros((512, 127), np.float32)], axis=1)
        weights[k + "_bf"] = np.ascontiguousarray(wv.astype(BF16))
    for k in BSHAPES:
        weights[k + "_f"] = np.ascontiguousarray(
            np.asarray(inputs[k], np.float32).reshape(-1, 1))
    meta, in_maps = _prep(x, edge_index, batch, sf, weights)
    meta["bo"] = float(np.asarray(inputs["bo"]).reshape(-1)[0])
    nc = _build(meta)
    res = bass_utils.run_bass_kernel_spmd(nc, in_maps, core_ids=list(range(W)),
                                          trace=True)
    out = np.zeros((NG, 1), np.float32)
    for c in range(W):
        out[c * GPC:(c + 1) * GPC, 0] = res.results[c]["out"][0]
    return out, res



# revision 17
# speedup vs baseline: 1.2468x; 1.1643x over previous
"""GCNNet (3x GCNConv + mean/max pool + MLP head) on 8 Trainium2 NeuronCores.

Sharding: graph-parallel. 256 graphs -> 32 per core; each core owns its graphs'
nodes (dst side of every edge), padded to SLOT-aligned node slots so pooling
segments are fixed 128-column windows. Aggregation D^-1/2 (A+I') D^-1/2 @ h is
computed per 128-slot dst block as a chain of 128-edge scatter-matmuls:
  lhsT = h_pre rows gathered by indirect DMA from a replicated DRAM table,
  rhs  = per-chunk scatter matrix (dis[dst] at [edge, dstlocal]) -> PSUM.
Self-loop term dis^2*h is added at PSUM eviction. Feature-major transform
(W.T @ agg.T) + relu follows. Between convs the dis-scaled output is
PE-transposed to node-major and AllGathered (bf16) so every core can gather
any source row. Pooling = masked segmented reduce_sum/reduce_max; MLP head is
computed feature-major for the core's 32 graphs.

All data-dependent structure (chunk counts, scatter matrices, gather indices,
masks, 1/count) is computed on the host per call and baked into the build.
"""
import sys

sys.path.insert(0, "/opt/trn_rl_repo")

import numpy as np
import ml_dtypes

import concourse.bass as bass
import concourse.bacc as bacc
import concourse.mybir as mybir
import concourse.tile as tile
from concourse import bass_utils
from concourse.masks import make_identity

BF16 = ml_dtypes.bfloat16
F8H = ml_dtypes.float8_e4m3
P = 128
W = 8                      # cores
NG = 256                   # graphs
GPC = NG // W              # graphs per core
D = 128                    # node feature dim
SOLV = 512

RELU = mybir.ActivationFunctionType.Relu
COPY = mybir.ActivationFunctionType.Copy
F32 = mybir.dt.float32
BF = mybir.dt.bfloat16
I32 = mybir.dt.int32
I16 = mybir.dt.int16
F8 = mybir.dt.float8e4
TW = 256            # fp8 table row width (gather elem must be 256B)

WSHAPES = {"W1": (D, D), "W2": (D, 2 * D), "W3": (2 * D, 4 * D),
           "Wg1": (1024, 1024), "Wg2": (1024, 128), "Ws1": (512, 256),
           "Ws2": (256, 128), "Wf1": (256, 1024), "Wf2": (1024, 512),
           "Wo": (512, 128)}   # Wo zero-padded 1 -> 128 cols
BSHAPES = {"b1": D, "b2": 2 * D, "b3": 4 * D, "bg1": 1024, "bg2": 128,
           "bs1": 256, "bs2": 128, "bf1": 1024, "bf2": 512}


def _prep(x, edge_index, batch, solvent_fingerprint, weights):
    """Host-side structure computation. Returns (meta, per-core input maps)."""
    N = x.shape[0]
    src = np.asarray(edge_index[0], np.int64)
    dst = np.asarray(edge_index[1], np.int64)
    batch = np.asarray(batch, np.int64)

    deg = np.bincount(dst, minlength=N).astype(np.float64) + 1.0
    dis = (1.0 / np.sqrt(deg)).astype(np.float32)

    counts = np.bincount(batch, minlength=NG).astype(np.int64)
    maxn = max(int(counts.max()), 1)
    SLOT = max(((maxn + P - 1) // P) * P, P)
    NLOC = GPC * SLOT
    NBLK = NLOC // P
    NTAB = W * NLOC

    # node -> (core, local slot, global slot); rank = position within graph
    order = np.argsort(batch, kind="stable")
    rank = np.zeros(N, np.int64)
    start_of = np.zeros(NG + 1, np.int64)
    np.add.at(start_of[1:], batch, 1)
    start_of = np.cumsum(start_of)
    rank[order] = np.arange(N) - start_of[batch[order]]
    core_of = batch // GPC
    lslot = (batch - core_of * GPC) * SLOT + rank
    gslot = core_of * NLOC + lslot

    x_pre = np.asarray(x, np.float32) * dis[:, None]
    x_tab = np.zeros((NTAB, D), F8H)
    x_tab[gslot] = x_pre.astype(F8H)

    # edges grouped by (core, local dst block)
    e_core = core_of[dst]
    e_blk = lslot[dst] // P
    e_dstlocal = lslot[dst] % P
    e_key = e_core * NBLK + e_blk
    e_order = np.argsort(e_key, kind="stable")
    cb_counts = np.bincount(e_key, minlength=W * NBLK).reshape(W, NBLK)
    seg_start = np.concatenate([[0], np.cumsum(cb_counts.reshape(-1))])
    # dedup gathered src rows within each (core, block)
    uniq_all = {}
    u_counts = np.zeros((W, NBLK), np.int64)
    for c in range(W):
        for b in range(NBLK):
            k = c * NBLK + b
            ee = e_order[seg_start[k]:seg_start[k + 1]]
            if len(ee):
                uq, inv = np.unique(gslot[src[ee]], return_inverse=True)
                uniq_all[k] = (ee, uq, inv)
                u_counts[c, b] = len(uq)
    cnt = np.maximum((u_counts + P - 1) // P, 1).max(axis=0)   # chunks/block
    NCHUNK = int(cnt.sum())
    off = np.concatenate([[0], np.cumsum(cnt)]).astype(np.int64)

    src_idx = np.zeros((W, P, NCHUNK), np.int32)
    scatf = np.zeros((W, P, NCHUNK, P), np.float32)
    gx1 = np.zeros((W, P, NCHUNK, D), F8H)
    for c in range(W):
        for b in range(NBLK):
            k = c * NBLK + b
            if k not in uniq_all:
                continue
            ee, uq, inv = uniq_all[k]
            base = off[b]
            uch = np.arange(len(uq)) // P
            upp = np.arange(len(uq)) % P
            src_idx[c, upp, base + uch] = uq.astype(np.int32)
            gx1[c, upp, base + uch, :] = x_tab[uq]
            np.add.at(scatf[c], (inv % P, base + inv // P, e_dstlocal[ee]),
                      dis[dst[ee]])
    scat = scatf.astype(F8H)

    # dma_gather index layout: flat order is chunk-major (row i -> partition
    # i%128, chunk i//128), wrapped into 16 partitions (idx i at [i%16, i//16])
    # and replicated across the 8 Q7 cores' partition groups. int16.
    assert NTAB <= 32768, "dma_gather int16 indices"
    idx16 = np.zeros((W, P, NCHUNK * 8), np.int16)
    for c in range(W):
        flat = src_idx[c].T.reshape(-1)                  # [NCHUNK*128] chunk-major
        w16 = flat.reshape(-1, 16).T.astype(np.int16)    # [16, NCHUNK*8]
        idx16[c] = np.tile(w16, (8, 1))

    dis_loc = np.zeros((W, NLOC), np.float32)
    dis2_loc = np.zeros((W, NLOC), np.float32)
    mask_loc = np.zeros((W, NLOC), np.float32)
    xT_loc = np.zeros((W, D, NLOC), BF16)
    for c in range(W):
        m = core_of == c
        dis_loc[c, lslot[m]] = dis[m]
        dis2_loc[c, lslot[m]] = dis[m] ** 2
        mask_loc[c, lslot[m]] = 1.0
        xT_loc[c][:, lslot[m]] = x_pre[m].T.astype(BF16)

    inv_cnt = (1.0 / np.maximum(counts, 1)).astype(np.float32).reshape(W, GPC)
    sf = np.asarray(solvent_fingerprint, np.float32)
    sfT = sf.reshape(NG, SOLV).T.astype(BF16)

    meta = dict(SLOT=SLOT, NLOC=NLOC, NBLK=NBLK, NTAB=NTAB, NCHUNK=NCHUNK,
                cnt=[int(v) for v in cnt], off=[int(v) for v in off],
                scmax=int(cnt.max()))

    bb = np.broadcast_to
    in_maps = []
    for c in range(W):
        m = dict(
            gx1=np.ascontiguousarray(gx1[c]),
            idx16=np.ascontiguousarray(idx16[c]),
            scat=np.ascontiguousarray(scat[c]),
            xT_loc=np.ascontiguousarray(xT_loc[c]),
            disb=np.ascontiguousarray(bb(dis_loc[c][None, :], (P, NLOC)).astype(BF16)),
            dis2b=np.ascontiguousarray(bb(dis2_loc[c][None, :], (P, NLOC)).astype(BF16)),
            maskb=np.ascontiguousarray(bb(mask_loc[c][None, :], (P, NLOC)).astype(BF16)),
            invb=np.ascontiguousarray(bb(inv_cnt[c][None, :], (P, GPC)).astype(np.float32)),
            sfT=np.ascontiguousarray(sfT[:, c * GPC:(c + 1) * GPC]),
        )
        m.update(weights)
        in_maps.append(m)
    return meta, in_maps


def _build(meta, trace=False):
    import os
    PH = int(os.environ.get("KPH", "9"))
    SLOT, NLOC, NBLK, NTAB, NCHUNK = (meta["SLOT"], meta["NLOC"], meta["NBLK"],
                                      meta["NTAB"], meta["NCHUNK"])
    cnt, off, scmax = meta["cnt"], meta["off"], meta["scmax"]
    CPG = 8            # chunks per gather call (SWDGE ring: <=1024 descs/inst)
    NGRP = (NCHUNK + CPG - 1) // CPG

    nc = bacc.Bacc("TRN2", target_bir_lowering=False, debug=False, num_devices=W,
                   num_swdge_queues=4)

    gx1 = nc.dram_tensor("gx1", [P, NCHUNK, D], F8, kind="ExternalInput")
    idx16 = nc.dram_tensor("idx16", [P, NCHUNK * 8], I16, kind="ExternalInput")
    scat = nc.dram_tensor("scat", [P, NCHUNK, P], F8, kind="ExternalInput")
    xT_loc = nc.dram_tensor("xT_loc", [D, NLOC], BF, kind="ExternalInput")
    disb = nc.dram_tensor("disb", [P, NLOC], BF, kind="ExternalInput")
    dis2b = nc.dram_tensor("dis2b", [P, NLOC], BF, kind="ExternalInput")
    maskb = nc.dram_tensor("maskb", [P, NLOC], BF, kind="ExternalInput")
    invb = nc.dram_tensor("invb", [P, GPC], F32, kind="ExternalInput")
    sfT = nc.dram_tensor("sfT", [SOLV, GPC], BF, kind="ExternalInput")

    wdr = {k: nc.dram_tensor(k + "_bf", list(s), BF, kind="ExternalInput")
           for k, s in WSHAPES.items()}
    bdr = {k: nc.dram_tensor(k + "_f", [n, 1], F32, kind="ExternalInput")
           for k, n in BSHAPES.items()}
    out = nc.dram_tensor("out", [1, GPC], F32, kind="ExternalOutput")

    with tile.TileContext(nc) as tc:
        with tc.tile_pool(name="const", bufs=1) as cp, \
             tc.tile_pool(name="gath", bufs=2) as gp, \
             tc.tile_pool(name="scatp", bufs=2) as sp, \
             tc.tile_pool(name="selfp", bufs=2) as selfp, \
             tc.tile_pool(name="aggp", bufs=2) as aggp, \
             tc.tile_pool(name="hp", bufs=6) as hp, \
             tc.tile_pool(name="work", bufs=2) as wp, \
             tc.tile_pool(name="psA", bufs=2, space="PSUM") as psA, \
             tc.tile_pool(name="psB", bufs=2, space="PSUM") as psB, \
             tc.tile_pool(name="psT", bufs=2, space="PSUM") as psT, \
             tc.tile_pool(name="dram", bufs=1, space="DRAM") as dp:

            # ---- constants ----
            ident = cp.tile([P, P], BF)
            make_identity(nc, ident[:])
            w_sb = {}
            for name, (fi, fo) in WSHAPES.items():
                tiles = []
                for k in range(fi // P):
                    t = cp.tile([P, fo], BF, name=f"w_{name}_{k}")
                    nc.sync.dma_start(out=t[:], in_=wdr[name][k * P:(k + 1) * P, :])
                    tiles.append(t)
                w_sb[name] = tiles
            b_sb = {}
            for name, n in BSHAPES.items():
                tiles = []
                for m in range(n // P):
                    t = cp.tile([P, 1], F32, name=f"b_{name}_{m}")
                    nc.sync.dma_start(out=t[:], in_=bdr[name][m * P:(m + 1) * P, :])
                    tiles.append(t)
                b_sb[name] = tiles
            idx16_sb = cp.tile([P, NCHUNK * 8], I16)
            nc.sync.dma_start(out=idx16_sb[:], in_=idx16[:])
            disb_sb = cp.tile([P, NLOC], BF)
            nc.sync.dma_start(out=disb_sb[:], in_=disb[:])
            dis2b_sb = cp.tile([P, NLOC], BF)
            nc.sync.dma_start(out=dis2b_sb[:], in_=dis2b[:])
            maskb_sb = cp.tile([P, NLOC], BF)
            nc.sync.dma_start(out=maskb_sb[:], in_=maskb[:])
            invb_sb = cp.tile([P, GPC], F32)
            nc.sync.dma_start(out=invb_sb[:], in_=invb[:])
            sfT_sb = []
            for k in range(SOLV // P):
                t = cp.tile([P, GPC], BF, name=f"sfT_{k}")
                nc.sync.dma_start(out=t[:], in_=sfT[k * P:(k + 1) * P, :])
                sfT_sb.append(t)
            xT_sb = hp.tile([P, NLOC], BF, tag="hT", name="xT_sb")
            nc.sync.dma_start(out=xT_sb[:], in_=xT_loc[:])

            def conv(tag, table, f_in, f_out, w_name, b_name, hprevT,
                     pregathered=False, post_slice=None):
                """-> list of f_out//128 SBUF tiles [128, NLOC] bf16 (h.T)."""
                kt, mt = f_in // P, f_out // P
                selft = []
                for k in range(kt):
                    st = selfp.tile([P, NLOC], BF, tag="selft",
                                    name=f"self_{tag}_{k}")
                    nc.vector.tensor_tensor(out=st[:], in0=hprevT[k],
                                            in1=dis2b_sb[:],
                                            op=mybir.AluOpType.mult)
                    selft.append(st)
                aggT = [aggp.tile([P, NLOC], BF, tag="aggT",
                                  name=f"agg_{tag}_{k}") for k in range(kt)]
                with nc.named_scope(f"gath_{tag}"):
                    gtiles, stiles = {}, {}

                    def issue_group(g):
                        ch0, ch1 = g * CPG, min((g + 1) * CPG, NCHUNK)
                        nch = ch1 - ch0
                        if pregathered:
                            gall = gp.tile([P, CPG, f_in], F8, tag="gallx",
                                           name="gallx", bufs=4)
                            nc.sync.dma_start(out=gall[:, :nch, :],
                                              in_=table[:, ch0:ch1, :])
                        else:
                            gall = gp.tile([P, CPG, TW], F8, tag="gall",
                                           name="gall", bufs=4)
                            nc.gpsimd.dma_gather(
                                gall[:, :nch, :], table[:],
                                idx16_sb[:, ch0 * 8:ch1 * 8],
                                nch * P, nch * P, TW, queue_num=g % 4)
                        st = sp.tile([P, CPG * P], F8, tag="scat",
                                     name="scat_sb", bufs=4)
                        nc.sync.dma_start(out=st[:, :nch * P],
                                          in_=scat[:, ch0:ch1, :])
                        gtiles[g] = gall
                        stiles[g] = st

                    for b in range(NBLK):
                        cb = cnt[b]
                        for g in range(off[b] // CPG,
                                       (off[b] + cb - 1) // CPG + 1):
                            if g not in gtiles:
                                issue_group(g)
                        ps = [psA.tile([P, P], F32, space="PSUM",
                                       tag=f"ps{k}", name=f"ps{k}")
                              for k in range(kt)]
                        for c in range(cb):
                            ch = off[b] + c
                            g, col = ch // CPG, ch % CPG
                            for k in range(kt):
                                nc.tensor.matmul(
                                    out=ps[k][:],
                                    lhsT=gtiles[g][:, col, k * P:(k + 1) * P],
                                    rhs=stiles[g][:, col * P:(col + 1) * P],
                                    start=(c == 0), stop=(c == cb - 1))
                        for k in range(kt):
                            nc.vector.tensor_tensor(
                                out=aggT[k][:, b * P:(b + 1) * P],
                                in0=ps[k][:],
                                in1=selft[k][:, b * P:(b + 1) * P],
                                op=mybir.AluOpType.add)
                hT = [hp.tile([P, NLOC], BF, tag="hT", name=f"h_{tag}_{m}")
                      for m in range(mt)]
                with nc.named_scope(f"xf_{tag}"):
                    for m in range(mt):
                        for n0 in range(0, NLOC, 512):
                            ps2 = psB.tile([P, 512], F32, space="PSUM",
                                           tag="ps2", name="ps2")
                            for k in range(kt):
                                nc.tensor.matmul(
                                    out=ps2[:],
                                    lhsT=w_sb[w_name][k][:, m * P:(m + 1) * P],
                                    rhs=aggT[k][:, n0:n0 + 512],
                                    start=(k == 0), stop=(k == kt - 1))
                            nc.scalar.activation(
                                out=hT[m][:, n0:n0 + 512], in_=ps2[:], func=RELU,
                                bias=b_sb[b_name][m][:, 0:1])
                            if post_slice is not None:
                                post_slice(m, n0, hT[m])
                return hT

            def make_ag_sink(tag, f):
                ag_in = dp.tile([NLOC, TW], F8, name=f"agin_{tag}")
                ag_out = dp.tile([NTAB, TW], F8, addr_space="Shared",
                                 name=f"agout_{tag}")

                def post_slice(m, n0, hTm):
                    hpre = wp.tile([P, 512], BF, tag="hpre", name="hpre", bufs=3)
                    nc.vector.tensor_tensor(out=hpre[:], in0=hTm[:, n0:n0 + 512],
                                            in1=disb_sb[:, n0:n0 + 512],
                                            op=mybir.AluOpType.mult)
                    for j in range(4):
                        b = n0 // P + j
                        pst = psT.tile([P, P], BF, space="PSUM", tag="pst",
                                       name="pst")
                        nc.tensor.transpose(out=pst[:],
                                            in_=hpre[:, j * P:(j + 1) * P],
                                            identity=ident[:])
                        ev = wp.tile([P, P], F8, tag="ev", name="ev", bufs=3)
                        nc.scalar.activation(out=ev[:], in_=pst[:], func=COPY)
                        nc.sync.dma_start(
                            out=ag_in[b * P:(b + 1) * P, m * P:(m + 1) * P],
                            in_=ev[:])

                def finish():
                    with nc.named_scope(f"ag_{tag}"):
                        nc.gpsimd.collective_compute(
                            "AllGather", mybir.AluOpType.bypass,
                            replica_groups=[list(range(W))],
                            ins=[ag_in[:]], outs=[ag_out[:]])
                    return ag_out
                return post_slice, finish

            # ---- conv stack ----
            sink1, fin1 = make_ag_sink("t1", D)
            h1T = conv("c1", gx1, D, D, "W1", "b1", [xT_sb[:]],
                       pregathered=True, post_slice=sink1)
            tab1 = fin1()
            sink2, fin2 = make_ag_sink("t2", 2 * D)
            h2T = conv("c2", tab1, D, 2 * D, "W2", "b2",
                       [t[:] for t in h1T], post_slice=sink2)
            tab2 = fin2()

            def mask_slice(m, n0, hTm):
                nc.vector.tensor_tensor(out=hTm[:, n0:n0 + 512],
                                        in0=hTm[:, n0:n0 + 512],
                                        in1=maskb_sb[:, n0:n0 + 512],
                                        op=mybir.AluOpType.mult)
            h3T = conv("c3", tab2, 2 * D, 4 * D, "W3", "b3",
                       [t[:] for t in h2T], post_slice=mask_slice)

            # ---- pooling ----
            gapT = wp.tile([P, 4, GPC], F32, name="gapT")
            gmpT = wp.tile([P, 4, GPC], F32, name="gmpT")
            with nc.named_scope("pool"):
                for m in range(4):
                    seg3 = h3T[m][:].rearrange("p (g s) -> p g s", s=SLOT)
                    nc.vector.reduce_sum(out=gapT[:, m, :], in_=seg3,
                                         axis=mybir.AxisListType.X)
                    nc.vector.reduce_max(out=gmpT[:, m, :], in_=seg3,
                                         axis=mybir.AxisListType.X)

            # ---- head ----
            def dense(tag, rhs_aps, w_name, b_name, f_in, f_out):
                kt, mt = f_in // P, f_out // P
                outs = []
                for m in range(mt):
                    ps2 = psB.tile([P, GPC], F32, space="PSUM", tag="ps2",
                                   name="ps2")
                    for k in range(kt):
                        nc.tensor.matmul(
                            out=ps2[:],
                            lhsT=w_sb[w_name][k][:, m * P:(m + 1) * P],
                            rhs=rhs_aps[k], start=(k == 0), stop=(k == kt - 1))
                    o = wp.tile([P, GPC], BF, tag=f"hd_{tag}_{m}", name=f"hd_{tag}_{m}")
                    nc.scalar.activation(out=o[:], in_=ps2[:], func=RELU,
                                         bias=b_sb[b_name][m][:, 0:1])
                    outs.append(o[:])
                return outs

            with nc.named_scope("head"):
                gcat = []
                for m in range(4):
                    t = wp.tile([P, GPC], BF, tag=f"gap_{m}", name=f"gap_{m}")
                    nc.vector.tensor_tensor(out=t[:], in0=gapT[:, m, :],
                                            in1=invb_sb[:],
                                            op=mybir.AluOpType.mult)
                    gcat.append(t[:])
                for m in range(4):
                    t = wp.tile([P, GPC], BF, tag=f"gmp_{m}", name=f"gmp_{m}")
                    nc.vector.tensor_copy(out=t[:], in_=gmpT[:, m, :])
                    gcat.append(t[:])
                g1 = dense("g1", gcat, "Wg1", "bg1", 1024, 1024)
                g2 = dense("g2", g1, "Wg2", "bg2", 1024, 128)
                s1 = dense("s1", [t[:] for t in sfT_sb], "Ws1", "bs1", 512, 256)
                s2 = dense("s2", s1, "Ws2", "bs2", 256, 128)
                f1 = dense("f1", g2 + s2, "Wf1", "bf1", 256, 1024)
                f2 = dense("f2", f1, "Wf2", "bf2", 1024, 512)
                pso = psB.tile([P, GPC], F32, space="PSUM", tag="ps2", name="pso")
                for k in range(4):
                    nc.tensor.matmul(out=pso[:], lhsT=w_sb["Wo"][k][:],
                                     rhs=f2[k], start=(k == 0), stop=(k == 3))
                oo = wp.tile([1, GPC], F32, name="oo")
                nc.scalar.activation(out=oo[:], in_=pso[0:1, :], func=COPY,
                                     bias=float(meta["bo"]))
                nc.sync.dma_start(out=out[:], in_=oo[:])

    nc.compile()
    return nc


def kernel(**inputs):
    x = np.asarray(inputs["x"], np.float32)
    edge_index = np.asarray(inputs["edge_index"])
    batch = np.asarray(inputs["batch"])
    sf = np.asarray(inputs["solvent_fingerprint"], np.float32)

    weights = {}
    for k in WSHAPES:
        wv = np.asarray(inputs[k], np.float32)
        if k == "Wo":                       # pad [512,1] -> [512,128]
            wv = np.concatenate([wv, np.zeros((512, 127), np.float32)], axis=1)
        weights[k + "_bf"] = np.ascontiguousarray(wv.astype(BF16))
    for k in BSHAPES:
        weights[k + "_f"] = np.ascontiguousarray(
            np.asarray(inputs[k], np.float32).reshape(-1, 1))

    meta, in_maps = _prep(x, edge_index, batch, sf, weights)
    meta["bo"] = float(np.asarray(inputs["bo"]).reshape(-1)[0])
    nc = _build(meta)
    res = bass_utils.run_bass_kernel_spmd(nc, in_maps, core_ids=list(range(W)))
    out = np.zeros((NG, 1), np.float32)
    for c in range(W):
        out[c * GPC:(c + 1) * GPC, 0] = res.results[c]["out"][0]
    return out


# exposed for test.py: run with tracing and return (out, results)
def kernel_traced(**inputs):
    x = np.asarray(inputs["x"], np.float32)
    edge_index = np.asarray(inputs["edge_index"])
    batch = np.asarray(inputs["batch"])
    sf = np.asarray(inputs["solvent_fingerprint"], np.float32)
    weights = {}
    for k in WSHAPES:
        wv = np.asarray(inputs[k], np.float32)
        if k == "Wo":
            wv = np.concatenate([wv, np.zeros((512, 127), np.float32)], axis=1)
        weights[k + "_bf"] = np.ascontiguousarray(wv.astype(BF16))
    for k in BSHAPES:
        weights[k + "_f"] = np.ascontiguousarray(
            np.asarray(inputs[k], np.float32).reshape(-1, 1))
    meta, in_maps = _prep(x, edge_index, batch, sf, weights)
    meta["bo"] = float(np.asarray(inputs["bo"]).reshape(-1)[0])
    nc = _build(meta)
    res = bass_utils.run_bass_kernel_spmd(nc, in_maps, core_ids=list(range(W)),
                                          trace=True)
    out = np.zeros((NG, 1), np.float32)
    for c in range(W):
        out[c * GPC:(c + 1) * GPC, 0] = res.results[c]["out"][0]
    return out, res

